# revision 26
# baseline (speedup 1.0000x reference)
"""CRF forward (logsumexp over paths) loss kernel for Trainium2, 8 NeuronCores.

Time-parallel chunked algorithm
-------------------------------
The linear-space recurrence  w_t = (ETs^T w_{t-1}) * e_t  (ETs = exp(trans-D),
e_t = exp(emit_t), state [K, B] per core) is a product of positive matrices,
so it forgets its initial condition at the Birkhoff contraction rate —
measured here at ~2 decades per 2 steps.  That lets the T=512 serial chain be
cut into S=32 time chunks run CONCURRENTLY: each chunk starts from the
uniform state w := e_{t0} a couple of steps (m=2) before its real range and
is correct in *direction* by the time the range starts; its unknown per-batch
log-magnitude offset delta_c is recovered afterwards by matching log-colsums
with the previous chunk at the shared boundary step (a tiny scalar cumsum).

Per core (64-batch shard), the 32 chunks run as 2 pair-groups of 16 batched
into the free axis: two [65, 512] matmuls per pair-step (65th weight column
of ones emits the colsum row Z for free) land in one [65, 1024] PSUM tile,
consumed by a single DVE multiply whose emission operand has a preset ones
row — so Z rides through into the persistent SBUF state ring and is
harvested by ONE gather-DMA per pair after the chain (GPSIMD DMA triggers
cost ~800ns each, so DMA count is minimized everywhere: emissions are
host-prearranged into the exact staging layout and load as two big
contiguous DMAs per pair-window).  Emissions are exp'd on the Scalar engine.
Final combine: per-chunk Z histories are PE-transposed to [b, slot], matched
into delta_c (log-ratio cumsum), and the one-hot time mask (host-preprocessed
into one-hot (chunk,slot) + chunk indicators) selects
ln Z(tau_b) + delta_c(b) + D*tau_b; a ones-matmul reduces the batch on core.

Sharding: batch 512 = 8 cores x 64, transitions/alpha_0 replicated; host sums
the 8 per-core scalars.
"""

import os
import sys

for _p in ("/opt/trn_rl_repo", "/root/.axon_site/_ro/trn_rl_repo"):
    if os.path.isdir(_p) and _p not in sys.path:
        sys.path.insert(0, _p)

from contextlib import ExitStack

import numpy as np

import concourse.bass as bass
import concourse.mybir as mybir
import concourse.tile as tile
from concourse.bass_utils import run_bass_kernel_spmd
from concourse.masks import make_identity

# Walrus in this container rejects instructions with >1 sync-wait; split the
# extras onto preceding same-engine no-ops (queues are in-order, so identical
# semantics).
_ORIG_COMMIT = tile.TileContext._commit_instruction


def _single_wait_commit(self, inst, lazy_reg_writes=True):
    si = getattr(inst, "sync_info", None)
    if (
        si is not None
        and si.on_wait
        and len(si.on_wait) > 1
        and inst.engine != mybir.EngineType.Unassigned
    ):
        waits = list(si.on_wait)
        eng = self.nc.engines[inst.engine]
        for w in waits[:-1]:
            n = eng.nop(nofuse=True)
            n.ins.sync_info = mybir.SyncInfo(on_wait=[w], on_update=[])
        inst.sync_info = mybir.SyncInfo(
            on_wait=[waits[-1]], on_update=list(si.on_update or [])
        )
    _ORIG_COMMIT(self, inst, lazy_reg_writes)


tile.TileContext._commit_instruction = _single_wait_commit

T, B, K = 512, 512, 64
NCORES = 8
BSH = B // NCORES      # 64 batch per core
P = 16                 # real steps per chunk
M = 1                  # burn-in steps
S = T // P             # 32 chunks
LL = P + M             # 17 chain steps per chunk (zbuf rows 1..LL)
NR = LL + 1            # 18 zbuf rows (row 0 unused, kept 1.0)
NR2 = NR              # 18: zT column pitch (even -> 4-byte aligned bf16 PSUM)
NP = 2                 # pair-groups
GP = S // NP           # 16 chunks per pair-group
PC = GP * BSH          # 1024 columns per pair-group
HC = PC // 2           # 512 columns per matmul
W = 3                  # emission window (steps per DMA/exp block)
NW = 6                 # windows cover slots 0..17 (slot 17 is zero padding)
DELTA = 4.0            # per-step log-space offset folded into ETs
F32 = mybir.dt.float32
BF16 = mybir.dt.bfloat16
MULT = mybir.AluOpType.mult
ADD = mybir.AluOpType.add
SUB = mybir.AluOpType.subtract
AX = mybir.AxisListType.X
AF = mybir.ActivationFunctionType


def _t_start(c):
    return 0 if c == 0 else c * P - M


def _build_crf_nc() -> bass.Bass:
    nc = bass.Bass(trn_type="TRN2", target_bir_lowering=False, debug=False)

    # emissions host-prearranged into per-(pair,window) staging blocks:
    # row (p*NW + wv)*K + k, col = step_in_window*PC + chunk_in_pair*BSH + b
    # 65th row is 0.0 so exp() yields the ones row for Z passthrough
    emt_d = nc.dram_tensor(
        "emits_blk", [NP * NW * (K + 1), W * PC], BF16, kind="ExternalInput"
    ).ap()
    trans_d = nc.dram_tensor("transitions", [K, K], F32, kind="ExternalInput").ap()
    alpha0_d = nc.dram_tensor("alpha_0", [K, 1], F32, kind="ExternalInput").ap()
    ohz_d = nc.dram_tensor("onehot_z", [BSH, S * NR2], F32, kind="ExternalInput").ap()
    ohc_d = nc.dram_tensor("onehot_c", [BSH, S], F32, kind="ExternalInput").ap()
    taud_d = nc.dram_tensor("tau_delta", [BSH, 1], F32, kind="ExternalInput").ap()
    out_d = nc.dram_tensor("out_sum", [1, 1], F32, kind="ExternalOutput").ap()

    with tile.TileContext(nc) as tc:
        with ExitStack() as ctx:
            _crf_body(ctx, tc, emt_d, trans_d, alpha0_d, ohz_d, ohc_d, taud_d,
                      out_d)
    _split_remaining_multiwaits(nc)
    return nc


def _split_remaining_multiwaits(nc):
    for blk in nc.m.functions[0].blocks:
        il = blk.instructions
        idx = 0
        while idx < len(il):
            inst = il[idx]
            si = inst.sync_info
            if si is not None and si.on_wait and len(si.on_wait) > 1:
                waits = list(si.on_wait)
                for j, w in enumerate(waits[:-1]):
                    n = mybir.InstNoOp(
                        name=f"I-swx-{inst.name}-{j}", ins=[], outs=[]
                    )
                    n.engine = inst.engine
                    n.sync_info = mybir.SyncInfo(on_wait=[w], on_update=[])
                    nc.register_instruction(n, overwrite=True)
                    il.insert(idx, n)
                    idx += 1
                inst.sync_info = mybir.SyncInfo(
                    on_wait=[waits[-1]], on_update=list(si.on_update or [])
                )
            idx += 1


def _crf_body(ctx, tc, emt_d, trans_d, alpha0_d, ohz_d, ohc_d, taud_d, out_d):
    nc = tc.nc

    # ---- long-lived SBUF ----
    ets = nc.alloc_sbuf_tensor("ets", [K, K + 1], BF16).ap()
    expal = nc.alloc_sbuf_tensor("expal", [K + 1, 1], F32).ap()
    identf = nc.alloc_sbuf_tensor("identf", [NR + 1, NR + 1], BF16).ap()
    ones_b = nc.alloc_sbuf_tensor("ones_b", [BSH, 1], F32).ap()
    cst = nc.alloc_sbuf_tensor("cst", [128, 2], F32).ap()  # col0=0, col1=-DELTA
    zbuf = [
        nc.alloc_sbuf_tensor(f"zbuf{p}", [NR, PC], BF16).ap() for p in range(NP)
    ]
    # state ring: slot s holds w_s [65, PC]; row 64 = Z(s-1) passthrough,
    # harvested by one gather-DMA per pair after the chain.
    wring = [
        nc.alloc_sbuf_tensor(f"wring{p}", [K + 1, (LL + 1) * PC], BF16).ap()
        for p in range(NP)
    ]
    # emission staging: per pair 3 persistent buffers [65, W*PC] bf16 with
    # row 64 = 1.0 (preset once) so the 65-row multiply passes Z through.
    eexp = [
        [nc.alloc_sbuf_tensor(f"eexp{p}_{i}", [K + 1, W * PC], BF16).ap()
         for i in range(4)]
        for p in range(NP)
    ]

    # ---- one-time setup (gpsimd: keeps the DVE queue free at startup) ----
    nc.gpsimd.memset(cst[:, 0:1], 0.0)
    nc.gpsimd.memset(cst[:, 1:2], -DELTA)
    nc.gpsimd.memset(ones_b[:, :], 1.0)
    fin_pool = ctx.enter_context(tc.tile_pool(name="fin", bufs=1))

    fpsum = ctx.enter_context(tc.tile_pool(name="fpsum", bufs=2, space="PSUM"))
    with ExitStack() as chain_ctx:
        raw_pool = chain_ctx.enter_context(tc.tile_pool(name="raw", bufs=3))
        u_psum = chain_ctx.enter_context(
            tc.tile_pool(name="upsum", bufs=1, space="PSUM")
        )

        def load_window(p, wv, nsplit=4, nexp=1):
            rt = raw_pool.tile([K + 1, W * PC], BF16, tag=f"raw{p}")
            r0 = (p * NW + wv) * (K + 1)
            q = W * PC // nsplit
            for i in range(nsplit):
                eng = nc.gpsimd if i % 2 == 0 else nc.sync
                eng.dma_start(
                    rt[:, i * q : (i + 1) * q],
                    emt_d[r0 : r0 + K + 1, i * q : (i + 1) * q],
                )
            dst = eexp[p][wv % 4]
            e = W * PC // nexp
            for i in range(nexp):
                nc.scalar.activation(
                    dst[:, i * e : (i + 1) * e],
                    rt[:, i * e : (i + 1) * e],
                    AF.Exp,
                    bias=cst[0 : K + 1, 0:1],
                )

        # transitions/alpha first: tiny loads whose exps gate the first MM
        a0_t = fin_pool.tile([K, 1], F32, tag="a0t")
        nc.gpsimd.dma_start(a0_t[:], alpha0_d)
        tr_t = fin_pool.tile([K, K], F32, tag="trt")
        nc.sync.dma_start(tr_t[:], trans_d)
        nc.scalar.activation(ets[:, 0:K], tr_t[:], AF.Exp, bias=cst[0:K, 1:2])
        nc.vector.memset(ets[:, K : K + 1], 1.0)
        nc.scalar.activation(expal[0:K], a0_t[:], AF.Exp, bias=cst[0:K, 0:1])
        nc.vector.memset(expal[K : K + 1], 1.0)
        for p in range(NP):
            load_window(p, 0, nsplit=8, nexp=3)
        for wv in range(1, 3):
            for p in range(NP):
                load_window(p, wv)
        # bulky one-time setup AFTER the loads so it never delays them
        for p in range(NP):
            nc.gpsimd.memset(zbuf[p][0:1, :], 1.0)  # row 0 -> ln = 0
        make_identity(nc, identf)

        # init states: w0 = e_{t0} (chunks >=1), chunk 0: expal * e_0
        for p in range(NP):
            wt = wring[p][:, 0:PC]
            sv = eexp[p][0][:, 0:PC]
            if p == 0:
                nc.vector.tensor_scalar(
                    wt[:, 0:BSH], sv[:, 0:BSH], expal, None, op0=MULT
                )
                nc.vector.tensor_copy(wt[:, BSH:PC], sv[:, BSH:PC])
            else:
                nc.vector.tensor_copy(wt[:, :], sv[:, :])
        for p in range(NP):
            load_window(p, 3, nsplit=6)

        # ---- chain: steps 1..LL ----
        for s in range(1, LL + 1):
            if s % W == 0 and s // W + 4 <= NW:
                for p in range(NP):
                    load_window(p, s // W + 3, nsplit=6)
            se = min(s, LL - 1)        # step LL reuses step LL-1's emission
            wv, sw = se // W, se % W
            for p in range(NP):
                u = u_psum.tile([K + 1, PC], F32, tag=f"u{p}")
                nc.tensor.matmul(
                    u[:, 0:HC],
                    ets[:, :],
                    wring[p][0:K, (s - 1) * PC : (s - 1) * PC + HC],
                    start=True,
                    stop=True,
                )
                nc.tensor.matmul(
                    u[:, HC:PC],
                    ets[:, :],
                    wring[p][0:K, (s - 1) * PC + HC : s * PC],
                    start=True,
                    stop=True,
                )
                if s < LL:
                    nc.vector.tensor_tensor(
                        wring[p][:, s * PC : (s + 1) * PC],
                        u[:, :],
                        eexp[p][wv % 4][:, sw * PC : (sw + 1) * PC],
                        op=MULT,
                    )
                else:
                    # final step only harvests Z(LL-1): copy u's colsum row
                    # into the ring on the (idle) Scalar engine
                    nc.scalar.copy(
                        wring[p][K : K + 1, s * PC : (s + 1) * PC],
                        u[K : K + 1, :],
                    )
                    # Z harvest: row 64 of slots 1..LL -> zbuf rows 1..LL
                    nc.gpsimd.dma_start(
                        zbuf[p][1 : LL + 1, :],
                        wring[p][K : K + 1, PC : (LL + 1) * PC].rearrange(
                            "r (s c) -> r s c", s=LL
                        ),
                    )


    # ---- final combine ----
    ohz = fin_pool.tile([BSH, S * NR2], F32, tag="ohz")
    nc.sync.dma_start(ohz[:], ohz_d)
    ohc = fin_pool.tile([BSH, S], F32, tag="ohc")
    nc.sync.dma_start(ohc[:], ohc_d)
    taud = fin_pool.tile([BSH, 1], F32, tag="taud")
    nc.sync.dma_start(taud[:], taud_d)
    zT = fin_pool.tile([BSH, S * NR2], F32, tag="zT")
    # pad columns would otherwise hold junk; preset whole tile Ln-safe
    nc.vector.memset(zT[:, :], 1.0)
    for h in range(2):
        zt = fpsum.tile([BSH, (S // 2) * NR2], BF16, tag="zt")
        for ci in range(S // 2):
            c = h * (S // 2) + ci
            p, gi = c // GP, c % GP
            nc.tensor.transpose(
                zt[:, ci * NR2 : ci * NR2 + NR],
                zbuf[p][:, gi * BSH : (gi + 1) * BSH],
                identf[0:NR, 0:NR],
            )
        nc.vector.tensor_copy(
            zT[:, h * (S // 2) * NR2 : (h + 1) * (S // 2) * NR2].rearrange(
                "b (c r) -> b c r", r=NR2
            )[:, :, 0:NR],
            zt[:].rearrange("b (c r) -> b c r", r=NR2)[:, :, 0:NR],
        )
    # patch: chunk0's matching column (row LL) := its row P (t = P-1)
    nc.vector.tensor_copy(zT[:, LL : LL + 1], zT[:, P : P + 1])
    lnz = fin_pool.tile([BSH, S * NR2], F32, tag="lnz")
    nc.scalar.activation(lnz[:], zT[:], AF.Ln, bias=cst[0:BSH, 0:1])

    # delta stitching: inc[:, i] = lnz[:, NR2*(i-1) + LL] - lnz[:, NR2*i + M]
    lv = lnz[:].rearrange("b (c r) -> b c r", r=NR2)
    inc = fin_pool.tile([BSH, S], F32, tag="inc")
    nc.vector.memset(inc[:, 0:1], 0.0)
    nc.vector.tensor_tensor(
        inc[:, 1:S], lv[:, 0 : S - 1, LL], lv[:, 1:S, M], op=SUB
    )
    scr1 = fin_pool.tile([BSH, S * NR2], F32, tag="scr1")
    zsel = fin_pool.tile([BSH, 1], F32, tag="zsel")
    nc.vector.tensor_tensor(scr1[:], lnz[:], ohz[:], op=MULT)
    nc.vector.tensor_reduce(zsel[:], scr1[:], axis=AX, op=ADD)
    # ohc is a step mask (1 for c <= chunk(tau_b)), so the cumulative-sum
    # of boundary increments folds into this single select-reduce.
    scr2 = fin_pool.tile([BSH, S], F32, tag="scr2")
    dsel = fin_pool.tile([BSH, 1], F32, tag="dsel")
    nc.vector.tensor_tensor(scr2[:], inc[:], ohc[:], op=MULT)
    nc.vector.tensor_reduce(dsel[:], scr2[:], axis=AX, op=ADD)
    res = fin_pool.tile([BSH, 1], F32, tag="res")
    nc.vector.tensor_tensor(res[:], zsel[:], dsel[:], op=ADD)
    nc.vector.tensor_tensor(res[:], res[:], taud[:], op=ADD)
    acc = fpsum.tile([1, 1], F32, tag="acc", bufs=1)
    nc.tensor.matmul(acc[:], res[:], ones_b[:], start=True, stop=True)
    osb = fin_pool.tile([1, 1], F32, tag="osb")
    nc.scalar.copy(osb[:], acc[:])
    nc.sync.dma_start(out_d, osb[:])


_NC_CACHE = None


def _get_nc():
    global _NC_CACHE
    if _NC_CACHE is None:
        _NC_CACHE = _build_crf_nc()
    return _NC_CACHE


def _make_in_maps(np_inputs):
    import ml_dtypes

    emits = np.asarray(np_inputs["emits"], dtype=np.float32)
    mask = np.asarray(np_inputs["mask"])
    transitions = np.asarray(np_inputs["transitions"], dtype=np.float32)
    alpha_0 = np.asarray(np_inputs["alpha_0"], dtype=np.float32)
    emits_t = emits.transpose(0, 2, 1)  # [T, K, B] view
    tau = mask.argmax(0).astype(np.int64)  # [B]
    chunk = tau // P
    row = np.where(chunk == 0, tau + 1, tau % P + M + 1)
    in_maps = []
    for cix in range(NCORES):
        sl = slice(cix * BSH, (cix + 1) * BSH)
        tau_s, c_s, r_s = tau[sl], chunk[sl], row[sl]
        ohz = np.zeros((BSH, S * NR2), dtype=np.float32)
        ohz[np.arange(BSH), c_s * NR2 + r_s] = 1.0
        ohc = (np.arange(S)[None, :] <= c_s[:, None]).astype(np.float32)
        taud = (DELTA * tau_s).astype(np.float32).reshape(BSH, 1)
        sh = emits_t[:, :, sl]  # [T, K, 64]
        # staging blocks [pair, window, k(+zero row), step, chunk_in_pair, b]
        nslot = NW * W
        blk = np.zeros((NP, NW, K + 1, W, GP, BSH), dtype=np.float32)
        for p in range(NP):
            for ci in range(GP):
                t0 = _t_start(p * GP + ci)
                ns = min(nslot, T - t0)
                sv = np.zeros((nslot, K, BSH), dtype=np.float32)
                sv[:ns] = sh[t0 : t0 + ns]
                blk[p, :, 0:K, :, ci, :] = (
                    sv.reshape(NW, W, K, BSH).transpose(0, 2, 1, 3)
                )
        emb = blk.reshape(NP * NW * (K + 1), W * PC).astype(ml_dtypes.bfloat16)
        in_maps.append(
            {
                "emits_blk": emb,
                "transitions": transitions,
                "alpha_0": alpha_0,
                "onehot_z": ohz,
                "onehot_c": ohc,
                "tau_delta": taud,
            }
        )
    return in_maps


def kernel(emits, mask, transitions, alpha_0):
    nc = _get_nc()
    in_maps = _make_in_maps(
        {"emits": emits, "mask": mask, "transitions": transitions,
         "alpha_0": alpha_0}
    )
    res = run_bass_kernel_spmd(nc, in_maps, core_ids=list(range(NCORES)))
    total = np.float64(0.0)
    for r in res.results:
        total += np.asarray(r["out_sum"], dtype=np.float64).sum()
    return np.float32(total)


# revision 27
# speedup vs baseline: 1.1801x; 1.1801x over previous
"""CRF forward (logsumexp over paths) loss kernel for Trainium2, 8 NeuronCores.

Time-parallel chunked algorithm
-------------------------------
The linear-space recurrence  w_t = (ETs^T w_{t-1}) * e_t  (ETs = exp(trans-D),
e_t = exp(emit_t), state [K, B] per core) is a product of positive matrices,
so it forgets its initial condition at the Birkhoff contraction rate —
measured here at ~2 decades per 2 steps.  That lets the T=512 serial chain be
cut into S=32 time chunks run CONCURRENTLY: each chunk starts from the
uniform state w := e_{t0} a couple of steps (m=2) before its real range and
is correct in *direction* by the time the range starts; its unknown per-batch
log-magnitude offset delta_c is recovered afterwards by matching log-colsums
with the previous chunk at the shared boundary step (a tiny scalar cumsum).

Per core (64-batch shard), the 32 chunks run as 2 pair-groups of 16 batched
into the free axis: two [65, 512] matmuls per pair-step (65th weight column
of ones emits the colsum row Z for free) land in one [65, 1024] PSUM tile,
consumed by a single DVE multiply whose emission operand has a preset ones
row — so Z rides through into the persistent SBUF state ring and is
harvested by ONE gather-DMA per pair after the chain (GPSIMD DMA triggers
cost ~800ns each, so DMA count is minimized everywhere: emissions are
host-prearranged into the exact staging layout and load as two big
contiguous DMAs per pair-window).  Emissions are exp'd on the Scalar engine.
Final combine: per-chunk Z histories are PE-transposed to [b, slot], matched
into delta_c (log-ratio cumsum), and the one-hot time mask (host-preprocessed
into one-hot (chunk,slot) + chunk indicators) selects
ln Z(tau_b) + delta_c(b) + D*tau_b; a ones-matmul reduces the batch on core.

Sharding: batch 512 = 8 cores x 64, transitions/alpha_0 replicated; host sums
the 8 per-core scalars.
"""

import os
import sys

for _p in ("/opt/trn_rl_repo", "/root/.axon_site/_ro/trn_rl_repo"):
    if os.path.isdir(_p) and _p not in sys.path:
        sys.path.insert(0, _p)

from contextlib import ExitStack

import numpy as np

import concourse.bass as bass
import concourse.mybir as mybir
import concourse.tile as tile
from concourse.bass_utils import run_bass_kernel_spmd
from concourse.masks import make_identity

# Walrus in this container rejects instructions with >1 sync-wait; split the
# extras onto preceding same-engine no-ops (queues are in-order, so identical
# semantics).
_ORIG_COMMIT = tile.TileContext._commit_instruction


def _single_wait_commit(self, inst, lazy_reg_writes=True):
    si = getattr(inst, "sync_info", None)
    if (
        si is not None
        and si.on_wait
        and len(si.on_wait) > 1
        and inst.engine != mybir.EngineType.Unassigned
    ):
        waits = list(si.on_wait)
        eng = self.nc.engines[inst.engine]
        for w in waits[:-1]:
            n = eng.nop(nofuse=True)
            n.ins.sync_info = mybir.SyncInfo(on_wait=[w], on_update=[])
        inst.sync_info = mybir.SyncInfo(
            on_wait=[waits[-1]], on_update=list(si.on_update or [])
        )
    _ORIG_COMMIT(self, inst, lazy_reg_writes)


tile.TileContext._commit_instruction = _single_wait_commit

T, B, K = 512, 512, 64
NCORES = 8
BSH = B // NCORES      # 64 batch per core
P = 16                 # real steps per chunk
M = 1                  # burn-in steps
S = T // P             # 32 chunks
LL = P + M             # 17 chain steps per chunk (zbuf rows 1..LL)
NR = LL + 1            # 18 zbuf rows (row 0 unused, kept 1.0)
NR2 = NR              # 18: zT column pitch (even -> 4-byte aligned bf16 PSUM)
NP = 2                 # pair-groups
GP = S // NP           # 16 chunks per pair-group
PC = GP * BSH          # 1024 columns per pair-group
HC = PC // 2           # 512 columns per matmul
W = 3                  # emission window (steps per DMA/exp block)
NW = 6                 # windows cover slots 0..17 (slot 17 is zero padding)
DELTA = 4.0            # per-step log-space offset folded into ETs
F32 = mybir.dt.float32
BF16 = mybir.dt.bfloat16
MULT = mybir.AluOpType.mult
ADD = mybir.AluOpType.add
SUB = mybir.AluOpType.subtract
AX = mybir.AxisListType.X
AF = mybir.ActivationFunctionType


def _t_start(c):
    return 0 if c == 0 else c * P - M


def _build_crf_nc() -> bass.Bass:
    nc = bass.Bass(trn_type="TRN2", target_bir_lowering=False, debug=False)

    # emissions host-prearranged into per-(pair,window) staging blocks:
    # row (p*NW + wv)*K + k, col = step_in_window*PC + chunk_in_pair*BSH + b
    # 65th row is 0.0 so exp() yields the ones row for Z passthrough
    emt_d = nc.dram_tensor(
        "emits_blk", [NP * NW * (K + 1), W * PC], BF16, kind="ExternalInput"
    ).ap()
    trans_d = nc.dram_tensor("transitions", [K, K], F32, kind="ExternalInput").ap()
    alpha0_d = nc.dram_tensor("alpha_0", [K, 1], F32, kind="ExternalInput").ap()
    ohz_d = nc.dram_tensor("onehot_z", [BSH, S * NR2], F32, kind="ExternalInput").ap()
    ohc_d = nc.dram_tensor("onehot_c", [BSH, S], F32, kind="ExternalInput").ap()
    taud_d = nc.dram_tensor("tau_delta", [BSH, 1], F32, kind="ExternalInput").ap()
    out_d = nc.dram_tensor("out_sum", [1, 1], F32, kind="ExternalOutput").ap()

    with tile.TileContext(nc) as tc:
        with ExitStack() as ctx:
            _crf_body(ctx, tc, emt_d, trans_d, alpha0_d, ohz_d, ohc_d, taud_d,
                      out_d)
    _split_remaining_multiwaits(nc)
    return nc


def _split_remaining_multiwaits(nc):
    for blk in nc.m.functions[0].blocks:
        il = blk.instructions
        idx = 0
        while idx < len(il):
            inst = il[idx]
            si = inst.sync_info
            if si is not None and si.on_wait and len(si.on_wait) > 1:
                waits = list(si.on_wait)
                for j, w in enumerate(waits[:-1]):
                    n = mybir.InstNoOp(
                        name=f"I-swx-{inst.name}-{j}", ins=[], outs=[]
                    )
                    n.engine = inst.engine
                    n.sync_info = mybir.SyncInfo(on_wait=[w], on_update=[])
                    nc.register_instruction(n, overwrite=True)
                    il.insert(idx, n)
                    idx += 1
                inst.sync_info = mybir.SyncInfo(
                    on_wait=[waits[-1]], on_update=list(si.on_update or [])
                )
            idx += 1


def _crf_body(ctx, tc, emt_d, trans_d, alpha0_d, ohz_d, ohc_d, taud_d, out_d):
    nc = tc.nc

    # ---- long-lived SBUF ----
    ets = nc.alloc_sbuf_tensor("ets", [K, K + 1], BF16).ap()
    expal = nc.alloc_sbuf_tensor("expal", [K + 1, 1], F32).ap()
    identf = nc.alloc_sbuf_tensor("identf", [NR + 1, NR + 1], BF16).ap()
    ones_b = nc.alloc_sbuf_tensor("ones_b", [BSH, 1], F32).ap()
    cst = nc.alloc_sbuf_tensor("cst", [128, 2], F32).ap()  # col0=0, col1=-DELTA
    zbuf = [
        nc.alloc_sbuf_tensor(f"zbuf{p}", [NR, PC], BF16).ap() for p in range(NP)
    ]
    # state ring: slot s holds w_s [65, PC]; row 64 = Z(s-1) passthrough,
    # harvested by one gather-DMA per pair after the chain.
    wring = [
        nc.alloc_sbuf_tensor(f"wring{p}", [K + 1, (LL + 1) * PC], BF16).ap()
        for p in range(NP)
    ]
    # emission staging: per pair 3 persistent buffers [65, W*PC] bf16 with
    # row 64 = 1.0 (preset once) so the 65-row multiply passes Z through.
    eexp = [
        [nc.alloc_sbuf_tensor(f"eexp{p}_{i}", [K + 1, W * PC], BF16).ap()
         for i in range(4)]
        for p in range(NP)
    ]

    # ---- one-time setup (gpsimd: keeps the DVE queue free at startup) ----
    nc.gpsimd.memset(cst[:, 0:1], 0.0)
    nc.gpsimd.memset(cst[:, 1:2], -DELTA)
    nc.gpsimd.memset(ones_b[:, :], 1.0)
    fin_pool = ctx.enter_context(tc.tile_pool(name="fin", bufs=1))

    fpsum = ctx.enter_context(tc.tile_pool(name="fpsum", bufs=2, space="PSUM"))
    with ExitStack() as chain_ctx:
        raw_pool = chain_ctx.enter_context(tc.tile_pool(name="raw", bufs=3))
        u_psum = chain_ctx.enter_context(
            tc.tile_pool(name="upsum", bufs=1, space="PSUM")
        )

        def load_window(p, wv, nsplit=4, nexp=1):
            rt = raw_pool.tile([K + 1, W * PC], BF16, tag=f"raw{p}")
            r0 = (p * NW + wv) * (K + 1)
            q = W * PC // nsplit
            for i in range(nsplit):
                eng = nc.gpsimd if i % 2 == 0 else nc.sync
                eng.dma_start(
                    rt[:, i * q : (i + 1) * q],
                    emt_d[r0 : r0 + K + 1, i * q : (i + 1) * q],
                )
            dst = eexp[p][wv % 4]
            e = W * PC // nexp
            for i in range(nexp):
                nc.scalar.activation(
                    dst[:, i * e : (i + 1) * e],
                    rt[:, i * e : (i + 1) * e],
                    AF.Exp,
                    bias=cst[0 : K + 1, 0:1],
                )

        # window-0 DMA triggers first; then the tiny trans/alpha loads whose
        # exps must precede the window exps in the ACT queue (they gate MM 1)
        a0_t = fin_pool.tile([K, 1], F32, tag="a0t")
        nc.gpsimd.dma_start(a0_t[:], alpha0_d)
        w0_tiles = []
        for p in range(NP):
            rt = raw_pool.tile([K + 1, W * PC], BF16, tag=f"raw{p}")
            r0 = (p * NW + 0) * (K + 1)
            q = W * PC // 8
            for i in range(8):
                eng = nc.gpsimd if i % 2 == 0 else nc.sync
                eng.dma_start(
                    rt[:, i * q : (i + 1) * q],
                    emt_d[r0 : r0 + K + 1, i * q : (i + 1) * q],
                )
            w0_tiles.append(rt)
        tr_t = fin_pool.tile([K, K], F32, tag="trt")
        nc.sync.dma_start(tr_t[:], trans_d)
        nc.scalar.activation(ets[:, 0:K], tr_t[:], AF.Exp, bias=cst[0:K, 1:2])
        nc.vector.memset(ets[:, K : K + 1], 1.0)
        nc.scalar.activation(expal[0:K], a0_t[:], AF.Exp, bias=cst[0:K, 0:1])
        nc.vector.memset(expal[K : K + 1], 1.0)
        for p in range(NP):
            dst = eexp[p][0]
            e = W * PC // 3
            for i in range(3):
                nc.scalar.activation(
                    dst[:, i * e : (i + 1) * e],
                    w0_tiles[p][:, i * e : (i + 1) * e],
                    AF.Exp,
                    bias=cst[0 : K + 1, 0:1],
                )
        for wv in range(1, 3):
            for p in range(NP):
                load_window(p, wv)
        # bulky one-time setup AFTER the loads so it never delays them
        for p in range(NP):
            nc.gpsimd.memset(zbuf[p][0:1, :], 1.0)  # row 0 -> ln = 0
        make_identity(nc, identf)

        # init states: w0 = e_{t0} (chunks >=1), chunk 0: expal * e_0
        for p in range(NP):
            wt = wring[p][:, 0:PC]
            sv = eexp[p][0][:, 0:PC]
            if p == 0:
                nc.vector.tensor_scalar(
                    wt[:, 0:BSH], sv[:, 0:BSH], expal, None, op0=MULT
                )
                nc.vector.tensor_copy(wt[:, BSH:PC], sv[:, BSH:PC])
            else:
                nc.vector.tensor_copy(wt[:, :], sv[:, :])
        for p in range(NP):
            load_window(p, 3, nsplit=6)

        # ---- chain: steps 1..LL ----
        for s in range(1, LL + 1):
            if s % W == 0 and s // W + 4 <= NW:
                for p in range(NP):
                    load_window(p, s // W + 3, nsplit=6)
            se = min(s, LL - 1)        # step LL reuses step LL-1's emission
            wv, sw = se // W, se % W
            for p in range(NP):
                u = u_psum.tile([K + 1, PC], F32, tag=f"u{p}")
                nc.tensor.matmul(
                    u[:, 0:HC],
                    ets[:, :],
                    wring[p][0:K, (s - 1) * PC : (s - 1) * PC + HC],
                    start=True,
                    stop=True,
                )
                nc.tensor.matmul(
                    u[:, HC:PC],
                    ets[:, :],
                    wring[p][0:K, (s - 1) * PC + HC : s * PC],
                    start=True,
                    stop=True,
                )
                if s < LL:
                    nc.vector.tensor_tensor(
                        wring[p][:, s * PC : (s + 1) * PC],
                        u[:, :],
                        eexp[p][wv % 4][:, sw * PC : (sw + 1) * PC],
                        op=MULT,
                    )
                else:
                    # final step only harvests Z(LL-1): copy u's colsum row
                    # into the ring on the (idle) Scalar engine
                    nc.scalar.copy(
                        wring[p][K : K + 1, s * PC : (s + 1) * PC],
                        u[K : K + 1, :],
                    )
                    # Z harvest: row 64 of slots 1..LL -> zbuf rows 1..LL
                    nc.gpsimd.dma_start(
                        zbuf[p][1 : LL + 1, :],
                        wring[p][K : K + 1, PC : (LL + 1) * PC].rearrange(
                            "r (s c) -> r s c", s=LL
                        ),
                    )


    # ---- final combine ----
    ohz = fin_pool.tile([BSH, S * NR2], F32, tag="ohz")
    nc.sync.dma_start(ohz[:], ohz_d)
    ohc = fin_pool.tile([BSH, S], F32, tag="ohc")
    nc.sync.dma_start(ohc[:], ohc_d)
    taud = fin_pool.tile([BSH, 1], F32, tag="taud")
    nc.sync.dma_start(taud[:], taud_d)
    zT = fin_pool.tile([BSH, S * NR2], F32, tag="zT")
    # pad columns would otherwise hold junk; preset whole tile Ln-safe
    nc.vector.memset(zT[:, :], 1.0)
    for h in range(2):
        zt = fpsum.tile([BSH, (S // 2) * NR2], BF16, tag="zt")
        for ci in range(S // 2):
            c = h * (S // 2) + ci
            p, gi = c // GP, c % GP
            nc.tensor.transpose(
                zt[:, ci * NR2 : ci * NR2 + NR],
                zbuf[p][:, gi * BSH : (gi + 1) * BSH],
                identf[0:NR, 0:NR],
            )
        nc.vector.tensor_copy(
            zT[:, h * (S // 2) * NR2 : (h + 1) * (S // 2) * NR2].rearrange(
                "b (c r) -> b c r", r=NR2
            )[:, :, 0:NR],
            zt[:].rearrange("b (c r) -> b c r", r=NR2)[:, :, 0:NR],
        )
    # patch: chunk0's matching column (row LL) := its row P (t = P-1)
    nc.vector.tensor_copy(zT[:, LL : LL + 1], zT[:, P : P + 1])
    lnz = fin_pool.tile([BSH, S * NR2], F32, tag="lnz")
    nc.scalar.activation(lnz[:], zT[:], AF.Ln, bias=cst[0:BSH, 0:1])

    # delta stitching: inc[:, i] = lnz[:, NR2*(i-1) + LL] - lnz[:, NR2*i + M]
    lv = lnz[:].rearrange("b (c r) -> b c r", r=NR2)
    inc = fin_pool.tile([BSH, S], F32, tag="inc")
    nc.vector.memset(inc[:, 0:1], 0.0)
    nc.vector.tensor_tensor(
        inc[:, 1:S], lv[:, 0 : S - 1, LL], lv[:, 1:S, M], op=SUB
    )
    scr1 = fin_pool.tile([BSH, S * NR2], F32, tag="scr1")
    zsel = fin_pool.tile([BSH, 1], F32, tag="zsel")
    nc.vector.tensor_tensor(scr1[:], lnz[:], ohz[:], op=MULT)
    nc.vector.tensor_reduce(zsel[:], scr1[:], axis=AX, op=ADD)
    # ohc is a step mask (1 for c <= chunk(tau_b)), so the cumulative-sum
    # of boundary increments folds into this single select-reduce.
    scr2 = fin_pool.tile([BSH, S], F32, tag="scr2")
    dsel = fin_pool.tile([BSH, 1], F32, tag="dsel")
    nc.vector.tensor_tensor(scr2[:], inc[:], ohc[:], op=MULT)
    nc.vector.tensor_reduce(dsel[:], scr2[:], axis=AX, op=ADD)
    res = fin_pool.tile([BSH, 1], F32, tag="res")
    nc.vector.tensor_tensor(res[:], zsel[:], dsel[:], op=ADD)
    nc.vector.tensor_tensor(res[:], res[:], taud[:], op=ADD)
    acc = fpsum.tile([1, 1], F32, tag="acc", bufs=1)
    nc.tensor.matmul(acc[:], res[:], ones_b[:], start=True, stop=True)
    osb = fin_pool.tile([1, 1], F32, tag="osb")
    nc.scalar.copy(osb[:], acc[:])
    nc.sync.dma_start(out_d, osb[:])


_NC_CACHE = None


def _get_nc():
    global _NC_CACHE
    if _NC_CACHE is None:
        _NC_CACHE = _build_crf_nc()
    return _NC_CACHE


def _make_in_maps(np_inputs):
    import ml_dtypes

    emits = np.asarray(np_inputs["emits"], dtype=np.float32)
    mask = np.asarray(np_inputs["mask"])
    transitions = np.asarray(np_inputs["transitions"], dtype=np.float32)
    alpha_0 = np.asarray(np_inputs["alpha_0"], dtype=np.float32)
    emits_t = emits.transpose(0, 2, 1)  # [T, K, B] view
    tau = mask.argmax(0).astype(np.int64)  # [B]
    chunk = tau // P
    row = np.where(chunk == 0, tau + 1, tau % P + M + 1)
    in_maps = []
    for cix in range(NCORES):
        sl = slice(cix * BSH, (cix + 1) * BSH)
        tau_s, c_s, r_s = tau[sl], chunk[sl], row[sl]
        ohz = np.zeros((BSH, S * NR2), dtype=np.float32)
        ohz[np.arange(BSH), c_s * NR2 + r_s] = 1.0
        ohc = (np.arange(S)[None, :] <= c_s[:, None]).astype(np.float32)
        taud = (DELTA * tau_s).astype(np.float32).reshape(BSH, 1)
        sh = emits_t[:, :, sl]  # [T, K, 64]
        # staging blocks [pair, window, k(+zero row), step, chunk_in_pair, b]
        nslot = NW * W
        blk = np.zeros((NP, NW, K + 1, W, GP, BSH), dtype=np.float32)
        for p in range(NP):
            for ci in range(GP):
                t0 = _t_start(p * GP + ci)
                ns = min(nslot, T - t0)
                sv = np.zeros((nslot, K, BSH), dtype=np.float32)
                sv[:ns] = sh[t0 : t0 + ns]
                blk[p, :, 0:K, :, ci, :] = (
                    sv.reshape(NW, W, K, BSH).transpose(0, 2, 1, 3)
                )
        emb = blk.reshape(NP * NW * (K + 1), W * PC).astype(ml_dtypes.bfloat16)
        in_maps.append(
            {
                "emits_blk": emb,
                "transitions": transitions,
                "alpha_0": alpha_0,
                "onehot_z": ohz,
                "onehot_c": ohc,
                "tau_delta": taud,
            }
        )
    return in_maps


def kernel(emits, mask, transitions, alpha_0):
    nc = _get_nc()
    in_maps = _make_in_maps(
        {"emits": emits, "mask": mask, "transitions": transitions,
         "alpha_0": alpha_0}
    )
    res = run_bass_kernel_spmd(nc, in_maps, core_ids=list(range(NCORES)))
    total = np.float64(0.0)
    for r in res.results:
        total += np.asarray(r["out_sum"], dtype=np.float64).sum()
    return np.float32(total)
